# revision 33
# baseline (speedup 1.0000x reference)
"""Conv1d-QKV + full attention kernel for TRN2, 8 NeuronCores — fp8 DoubleRow.

Problem (hardcoded): B=4, S=4096, DIN=DQ=DK=256.
  q = conv1d(query, q_w, q_b); k = conv1d(key, ...); v = conv1d(value, ...)
  out = scale * softmax(q @ k^T / sqrt(256)) @ v

Sharding: 8 cores = (batch b = core//2) x (query half h = core%2); each core
convs k/v over the full 4096 rows of its batch and runs attention for its
2048 query rows.

Numerics (validated vs reference in fp-sim + hw, scale_rel 1.55e-2 < 2e-2):
  - all matmuls fp8e4 DoubleRow (two 128-chunks folded per matmul, 0.5
    cycles/output-col in the cost model = 4x over f32r).
  - q/k convs: 2-term fp8 residual (w8x8 + w8xr8); v conv: 3-term
    (+ wr8x8) since v errors reach the output unaveraged.
  - scores: q,k quantized to fp8 at 2^6; exp on Act with the combined
    descale folded into the activation scale.
  - e path: centered quantization ec8 = fp8(32*(exp(s)-1)); the matching
    correction c*colsum(v8+vr8) is one replicated ones-stationary matmul,
    folded into each output PSUM group via a rank-1 f32r matmul.
  - v path: 2-term fp8 residual (v8 + vr8) of the f32 conv result; v bias
    enters the conv PSUM as a rank-1 f32r matmul (bias error would not
    average out through the softmax weights, so it stays f32); the global
    output scale rides the vtmp descale multiplier.
  - softmax denominator rides as an extra v_aug column (value 64 = v scale).

Engine split: exp on Act; (e-1)*32 center-casts distributed over DVE/GPSIMD
by CFG["ec_pat"]; v8 casts alternate Act/DVE; final scaling on DVE. Scores
pairs are interleaved into the conv/out emission so Act's exp stream and the
PE never starve each other; x tiles DMA on the Act HWDGE queue at startup
while weights use the SP queue.
"""

import numpy as np

B, S, DIN, D = 4, 4096, 256, 256
NCORES = 8
SQ = S // 2
NKT = S // 128        # 32 key tiles
NKP = NKT // 2        # 16 key-tile pairs
CT = 512              # conv column tile
XT_ST = 528           # conv x tile stride (mult of 16 for fp8 DoubleRow)
VW = 272              # v_aug row stride (mult of 16); data 0:256, den col 256
QB = 512              # query block
NBLK = SQ // QB       # 4

SX = 32.0             # x scale 2^5
SW = 4096.0           # w scale 2^12
SQK = 64.0            # q/k fp8 scale 2^6
SV = 64.0             # v fp8 scale 2^6
SE = 32.0             # e fp8 scale 2^5
EXP_SCALE = 1.0 / (16.0 * 4096.0)   # 1/16 softmax scale / (2^6*2^6 qk scale)

CFG = {
    # per-block engine patterns for the center-cast pass: A=Act, D=DVE, P=Pool
    # (chosen so each pipeline section stays below its pacing engine)
    "ec_pat": {
        0: "PPDPPDPPDPPDPPDP",
        1: "PPPPPPPPDPPPPPPP",
        2: "DDPDPDPDPDPDPDPD",
        3: "DDPDDPDPDDPDPDPD",
    },
    "ps_sc_bufs": 2,
    "ps_o_bufs": 2,
    "ps_cv_bufs": 2,
    "v8_mod": 2,
    "xin_bufs": 4,
    "et_bufs": 6,
}


def _split_drain_waits(nc):
    """Walrus accepts one sem-wait per CTRL (Drain) instruction; split any
    multi-wait Drain into a chain of single-wait no-ops."""
    import concourse.mybir as mybir

    def walk(blocks):
        for b in blocks:
            insts = b.instructions
            i = 0
            while i < len(insts):
                inst = insts[i]
                si = getattr(inst, "sync_info", None)
                w = list(si.on_wait) if si is not None and si.on_wait else []
                if len(w) > 1:
                    pre = [
                        mybir.InstNoOp(
                            name=f"{inst.name}-ws{j}",
                            engine=inst.engine,
                            ins=[],
                            outs=[],
                            sync_info=mybir.SyncInfo(on_wait=[wj], on_update=[]),
                        )
                        for j, wj in enumerate(w[:-1])
                    ]
                    si.on_wait = w[-1:]
                    for k, nd in enumerate(pre):
                        insts.insert(i + k, nd)
                    i += len(pre)
                i += 1
            walk(getattr(b, "blocks", []) or [])

    for f in nc.m.functions:
        walk(f.blocks)


def _build_bass():
    import concourse.bass as bass
    import concourse.mybir as mybir
    import concourse.tile as tile

    f32 = mybir.dt.float32
    f32r = mybir.dt.float32r
    bf16 = mybir.dt.bfloat16
    fp8 = mybir.dt.float8e4
    DR = mybir.MatmulPerfMode.DoubleRow
    ADD = mybir.AluOpType.add
    SUB = mybir.AluOpType.subtract
    MULT = mybir.AluOpType.mult
    Exp = mybir.ActivationFunctionType.Exp
    Copy = mybir.ActivationFunctionType.Copy

    nc = bass.Bass(trn_type="TRN2")

    # ---- DRAM I/O (per-core shard shapes) ----
    # x tiles: dim2 slots 0:2 = x8 (a-chunks), 2:4 = xr8
    xq = nc.dram_tensor("xq", [SQ // CT, 128, 4, XT_ST], fp8, kind="ExternalInput")
    xk = nc.dram_tensor("xk", [S // CT, 128, 4, XT_ST], fp8, kind="ExternalInput")
    xv = nc.dram_tensor("xv", [S // CT, 128, 4, XT_ST], fp8, kind="ExternalInput")
    # T-layout weights: [p, w/wr, a, t, o, m]
    wq = nc.dram_tensor("wq", [128, 2, 2, 3, 2, 128], fp8, kind="ExternalInput")
    wk = nc.dram_tensor("wk", [128, 2, 2, 3, 2, 128], fp8, kind="ExternalInput")
    # natural-layout v weights: [p, w/wr, a, t, co]
    wv = nc.dram_tensor("wv", [128, 2, 2, 3, D], fp8, kind="ExternalInput")
    bq = nc.dram_tensor("bq", [128, 2], f32, kind="ExternalInput")
    bk = nc.dram_tensor("bk", [128, 2], f32, kind="ExternalInput")
    bvrow = nc.dram_tensor("bvrow", [1, D], f32r, kind="ExternalInput")
    onesr = nc.dram_tensor("onesr", [1, 128], f32r, kind="ExternalInput")
    ones8 = nc.dram_tensor("ones8", [128, 2, 128], fp8, kind="ExternalInput")
    vtop8 = nc.dram_tensor("vtop8", [128, NKT, VW - D], fp8, kind="ExternalInput")
    vtop0 = nc.dram_tensor("vtop0", [128, NKT, VW - D], fp8, kind="ExternalInput")
    scl11 = nc.dram_tensor("scl11", [128, 1], f32, kind="ExternalInput")
    out = nc.dram_tensor("out", [SQ // 128, 128, D], f32, kind="ExternalOutput")

    ec_pat = CFG["ec_pat"]
    NQUAD = NKT // 4  # 8 score quads per block

    with tile.TileContext(nc) as tc:
        with (
            tc.tile_pool(name="persist", bufs=1) as persist,
            tc.tile_pool(name="xin", bufs=CFG["xin_bufs"]) as xin,
            tc.tile_pool(name="et", bufs=CFG["et_bufs"]) as etp,
            tc.tile_pool(name="vt", bufs=CFG.get("vt_bufs", 3)) as vtp,
            tc.tile_pool(name="outp", bufs=CFG.get("outp_bufs", 3)) as outp,
            tc.tile_pool(name="tiny", bufs=4) as tiny,
            tc.tile_pool(name="ps_cv", bufs=CFG["ps_cv_bufs"], space="PSUM") as ps_cv,
            tc.tile_pool(name="ps_sc", bufs=CFG["ps_sc_bufs"], space="PSUM") as ps_sc,
            tc.tile_pool(name="ps_o", bufs=CFG["ps_o_bufs"], space="PSUM") as ps_o,
        ):
            wq_s = persist.tile([128, 2, 2, 3, 2, 128], fp8, tag="wq_s")
            wk_s = persist.tile([128, 2, 2, 3, 2, 128], fp8, tag="wk_s")
            wv_s = persist.tile([128, 2, 2, 3, D], fp8, tag="wv_s")
            bq_s = persist.tile([128, 2], f32, tag="bq_s")
            bk_s = persist.tile([128, 2], f32, tag="bk_s")
            bvrow_s = persist.tile([1, D], f32r, tag="bvrow_s")
            onesr_s = persist.tile([1, 128], f32r, tag="onesr_s")
            ones8_s = persist.tile([128, 2, 128], fp8, tag="ones8_s")
            scl11_s = persist.tile([128, 1], f32, tag="scl11_s")
            qT8_s = persist.tile([128, 2, SQ], fp8, tag="qT8_s")
            kT8_s = persist.tile([128, 2, S], fp8, tag="kT8_s")
            v8_s = persist.tile([128, NKT, VW], fp8, tag="v8_s")
            vr8_s = persist.tile([128, NKT, VW], fp8, tag="vr8_s")
            corr_s = persist.tile([128, 260], f32r, tag="corr_s")
            ec8_s = persist.tile([128, NBLK, NKT, QB], fp8, tag="ec8_s")

            # critical-path DMAs only; the rest issue after q-conv emission.
            # wq split so the first matmul only waits on the w8 half; weights
            # go on the SP HWDGE queue while x tiles go on the Act queue so
            # the transfers overlap (Act's sequencer is idle this early).
            nc.sync.dma_start(wq_s[:, 0], wq[:, 0])
            nc.sync.dma_start(wq_s[:, 1], wq[:, 1])
            nc.sync.dma_start(bq_s[:], bq[:])

            # fp8 residual terms (w-residual, x-residual); qk convs run
            # 2-term (w8x8 + w8xr8, ~1.65e-2 total vs 2e-2 gate), the v conv
            # keeps all 3 terms since v errors reach the output unaveraged.

            TERMS_QK = [(0, 0), (0, 1)]
            TERMS_V = [(0, 0), (0, 1), (1, 0)]

            def conv_T(x_dram, w_s, b_s, out_s, nj, only_j=None, dma_eng=None):
                """T-layout conv; out_s[:, o, s] with co on partitions."""
                js = range(nj) if only_j is None else [only_j]
                for j in js:
                    xt = xin.tile([128, 4, XT_ST], fp8, tag="xt")
                    (dma_eng or nc.sync).dma_start(xt[:], x_dram[j])
                    for o in range(2):
                        ps = ps_cv.tile([128, CT], f32, tag="ps_cv")
                        n = len(TERMS_QK) * 3
                        i = 0
                        for rw, rx in TERMS_QK:
                            for t in range(3):
                                nc.tensor.matmul(
                                    ps[:],
                                    w_s[:, rw, :, t, o, :],
                                    xt[:, 2 * rx : 2 * rx + 2, t : t + CT],
                                    start=(i == 0),
                                    stop=(i == n - 1),
                                    perf_mode=DR,
                                )
                                i += 1
                        # out = ps * 2^-11 + bias*2^6 -> fp8
                        nc.vector.tensor_scalar(
                            out=out_s[:, o, j * CT : (j + 1) * CT],
                            in0=ps[:],
                            scalar1=2.0**-11,
                            scalar2=b_s[:, o : o + 1],
                            op0=MULT,
                            op1=ADD,
                        )

            def conv_v(j):
                """natural-layout v conv for x tile j -> v8/vr8 k-tiles."""
                xt = xin.tile([128, 4, XT_ST], fp8, tag="xt")
                nc.sync.dma_start(xt[:], xv[j])
                for r in range(CT // 128):
                    kt = j * (CT // 128) + r
                    ps = ps_cv.tile([128, CT], f32, tag="ps_cv")
                    first = True
                    for rw, rx in TERMS_V:
                        for t in range(3):
                            nc.tensor.matmul(
                                ps[:, 0:D],
                                xt[:, 2 * rx : 2 * rx + 2,
                                   t + r * 128 : t + r * 128 + 128],
                                wv_s[:, rw, :, t, :],
                                start=first,
                                stop=False,
                                perf_mode=DR,
                            )
                            first = False
                    # bias as rank-1 f32r (kept high precision: bias error
                    # would not average out through the softmax weights)
                    nc.tensor.matmul(ps[:, 0:D], onesr_s[:], bvrow_s[:],
                                     start=False, stop=True)
                    vtmp = vtp.tile([128, D], f32, tag="vtmp")
                    nc.vector.tensor_scalar(out=vtmp[:], in0=ps[:, 0:D],
                                            scalar1=scl11_s[:], scalar2=None,
                                            op0=MULT)
                    # v8 cast split Act/DVE (DVE is the pacing engine here)
                    if kt % CFG.get("v8_mod", 2) != 0:
                        nc.vector.tensor_scalar(out=v8_s[:, kt, 0:D],
                                                in0=vtmp[:], scalar1=0.0,
                                                scalar2=None, op0=ADD)
                    else:
                        nc.scalar.activation(out=v8_s[:, kt, 0:D],
                                             in_=vtmp[:], func=Copy)
                    nc.vector.tensor_tensor(out=vr8_s[:, kt, 0:D], in0=vtmp[:],
                                            in1=v8_s[:, kt, 0:D], op=SUB)

            def scores_pair(blk, kp):
                """scores^T for key tiles 2kp,2kp+1 vs query block blk,
                exp and center-cast to ec8_s."""
                q0 = blk * QB
                ps = ps_sc.tile([128, 2, QB], f32, tag="ps_sc")
                for i in range(2):
                    kt = 2 * kp + i
                    nc.tensor.matmul(
                        ps[:, i, :],
                        kT8_s[:, :, kt * 128 : (kt + 1) * 128],
                        qT8_s[:, :, q0 : q0 + QB],
                        start=True,
                        stop=True,
                        perf_mode=DR,
                    )
                et = etp.tile([128, 2, QB], bf16, tag="et")
                nc.scalar.activation(out=et[:], in_=ps[:], func=Exp,
                                     scale=EXP_SCALE)
                dst = ec8_s[:, blk, 2 * kp : 2 * kp + 2, :]
                eng = ec_pat[blk][kp]
                if eng == "A":
                    nc.scalar.activation(out=dst, in_=et[:], func=Copy,
                                         scale=SE, bias=-SE)
                elif eng == "P":
                    nc.gpsimd.tensor_scalar(out=dst, in0=et[:], scalar1=SE,
                                            scalar2=-SE, op0=MULT, op1=ADD)
                else:
                    nc.vector.tensor_scalar(out=dst, in0=et[:], scalar1=SE,
                                            scalar2=-SE, op0=MULT, op1=ADD)

            def out_tile(blk, qs):
                """po[q,:] = sum_k (ec8+c)(v8+vr8) with denominator column."""
                po = ps_o.tile([128, 260], f32, tag="ps_o")
                first = True
                for vs in (v8_s, vr8_s):
                    for kp in range(NKP):
                        nc.tensor.matmul(
                            po[:],
                            ec8_s[:, blk, 2 * kp : 2 * kp + 2,
                                  qs * 128 : (qs + 1) * 128],
                            vs[:, 2 * kp : 2 * kp + 2, 0:260],
                            start=first,
                            stop=False,
                            perf_mode=DR,
                        )
                        first = False
                nc.tensor.matmul(po[:], onesr_s[:], corr_s[0:1, :],
                                 start=False, stop=True)
                rec = tiny.tile([128, 1], f32, tag="rec")
                nc.vector.reciprocal(rec[:], po[:, 256:257])
                ot = outp.tile([128, D], f32, tag="ot")
                if blk in CFG.get("out_act_blks", ()):
                    nc.scalar.activation(out=ot[:], in_=po[:, 0:D], func=Copy,
                                         scale=rec[:])
                else:
                    nc.vector.tensor_scalar(out=ot[:], in0=po[:, 0:D],
                                            scalar1=rec[:], scalar2=None,
                                            op0=MULT)
                row = blk * (QB // 128) + qs
                nc.sync.dma_start(out[row], ot[:])

            # ---- emission order (software pipeline) ----
            # scores pairs are interleaved into the conv/out sections so the
            # Act exp stream never starves and never backs up the PE.
            conv_T(xq, wq_s, bq_s, qT8_s, SQ // CT, dma_eng=nc.scalar)
            nc.sync.dma_start(wk_s[:], wk[:])
            nc.sync.dma_start(bk_s[:], bk[:])
            for j in range(S // CT):
                # k-conv tile j, then score the two key-tile pairs it enables
                conv_T(xk, wk_s, bk_s, kT8_s, S // CT, only_j=j)
                scores_pair(0, 2 * j)
                scores_pair(0, 2 * j + 1)
            nc.sync.dma_start(wv_s[:], wv[:])
            nc.sync.dma_start(bvrow_s[:], bvrow[:])
            nc.sync.dma_start(onesr_s[:], onesr[:])
            nc.sync.dma_start(ones8_s[:], ones8[:])
            nc.sync.dma_start(scl11_s[:], scl11[:])
            nc.sync.dma_start(v8_s[:, :, D:VW], vtop8[:])
            nc.sync.dma_start(vr8_s[:, :, D:VW], vtop0[:])
            for j in range(S // CT):
                conv_v(j)
                scores_pair(1, 2 * j)
                scores_pair(1, 2 * j + 1)
            # correction: c * colsum(v8+vr8) (x2^5), replicated over rows
            pc = ps_o.tile([128, 260], f32, tag="ps_o")
            first = True
            for vs in (v8_s, vr8_s):
                for kp in range(NKP):
                    nc.tensor.matmul(pc[:], ones8_s[:],
                                     vs[:, 2 * kp : 2 * kp + 2, 0:260],
                                     start=first, stop=(vs is vr8_s and kp == NKP - 1),
                                     perf_mode=DR)
                    first = False
            nc.vector.tensor_scalar(out=corr_s[:], in0=pc[:], scalar1=SE,
                                    scalar2=None, op0=MULT)
            for i in range(4):
                scores_pair(2, 4 * i)
                scores_pair(2, 4 * i + 1)
                out_tile(0, i)
                scores_pair(2, 4 * i + 2)
                scores_pair(2, 4 * i + 3)
                out_tile(1, i)
            for i in range(4):
                scores_pair(3, 4 * i)
                scores_pair(3, 4 * i + 1)
                if i > 0:
                    out_tile(2, i - 1)
                scores_pair(3, 4 * i + 2)
                scores_pair(3, 4 * i + 3)
            out_tile(2, 3)
            for i in range(4):
                out_tile(3, i)

    _split_drain_waits(nc)
    return nc


_NC_CACHE = None


def _get_nc():
    global _NC_CACHE
    if _NC_CACHE is None:
        _NC_CACHE = _build_bass()
    return _NC_CACHE


def _fp8(a):
    import ml_dtypes
    return np.asarray(np.clip(a, -240.0, 240.0), ml_dtypes.float8_e4m3)


def _xtiles(x_pad):
    """[128, 2, n+2] f32 -> fp8 2-term tiles [nj, 128, 4, 528]."""
    n = x_pad.shape[2] - 2
    nj = n // CT
    x8 = _fp8(x_pad * SX)
    xr8 = _fp8(x_pad * SX - x8.astype(np.float32))
    tiles = np.zeros((nj, 128, 4, XT_ST), x8.dtype)
    for j in range(nj):
        sl = slice(j * CT, j * CT + CT + 2)
        tiles[j, :, 0:2, 0 : CT + 2] = x8[:, :, sl]
        tiles[j, :, 2:4, 0 : CT + 2] = xr8[:, :, sl]
    return tiles


def _xT_padded(x_b):
    """[S, C] -> transposed + halo-padded [128, 2, S+2] f32."""
    xt = np.zeros((DIN, x_b.shape[0] + 2), np.float32)
    xt[:, 1:-1] = x_b.T
    return np.ascontiguousarray(
        xt.reshape(2, 128, x_b.shape[0] + 2).transpose(1, 0, 2)
    )


def _w2(w_scaled):
    """scaled f32 weights -> (w8, wr8) fp8 pair."""
    w8 = _fp8(w_scaled)
    wr8 = _fp8(w_scaled - w8.astype(np.float32))
    return w8, wr8


def _prep_shared(q_w, q_b, k_w, k_b, v_w, v_b, scale):
    import ml_dtypes
    FP8 = ml_dtypes.float8_e4m3

    def w_T(w):  # [co, ci, 3] -> [p, a, t, o, m] f32
        arr = np.ascontiguousarray(w.transpose(1, 2, 0))  # [ci, t, co]
        arr = arr.reshape(2, 128, 3, 2, 128)  # [a, p, t, o, m]
        return np.ascontiguousarray(arr.transpose(1, 0, 2, 3, 4)).astype(np.float32)

    def w_v(w):  # [co, ci, 3] -> [p, a, t, co] f32
        arr = np.ascontiguousarray(w.transpose(1, 2, 0))
        arr = arr.reshape(2, 128, 3, D)
        return np.ascontiguousarray(arr.transpose(1, 0, 2, 3)).astype(np.float32)

    def pack_T(w):
        w8, wr8 = _w2(w_T(w) * SW)
        return np.ascontiguousarray(np.stack([w8, wr8], axis=1))

    wv8, wvr8 = _w2(w_v(v_w) * SW)
    vtop8 = np.zeros((128, NKT, VW - D), FP8)
    vtop8[:, :, 0] = FP8(SV)
    return {
        "wq": pack_T(q_w),
        "wk": pack_T(k_w),
        "wv": np.ascontiguousarray(np.stack([wv8, wvr8], axis=1)),
        "bq": np.ascontiguousarray(q_b.reshape(2, 128).T).astype(np.float32) * SQK,
        "bk": np.ascontiguousarray(k_b.reshape(2, 128).T).astype(np.float32) * SQK,
        "bvrow": v_b.astype(np.float32)[None, :] * (SW * SX),
        "onesr": np.ones((1, 128), np.float32),
        "ones8": np.ones((128, 2, 128), FP8),
        "vtop8": vtop8,
        "vtop0": np.zeros((128, NKT, VW - D), FP8),
        "scl11": np.full((128, 1), float(scale) * 2.0**-11, np.float32),
    }


def kernel(query, key, value, q_w, q_b, k_w, k_b, v_w, v_b, scale):
    from concourse.bass_utils import run_bass_kernel_spmd

    query = np.asarray(query, np.float32)
    key = np.asarray(key, np.float32)
    value = np.asarray(value, np.float32)

    shared = _prep_shared(
        np.asarray(q_w), np.asarray(q_b), np.asarray(k_w), np.asarray(k_b),
        np.asarray(v_w), np.asarray(v_b), np.asarray(scale),
    )

    in_maps = []
    for c in range(NCORES):
        b, h = c // 2, c % 2
        xq_full = _xT_padded(query[b])  # [128, 2, S+2]
        xq_c = np.ascontiguousarray(xq_full[:, :, h * SQ : h * SQ + SQ + 2])
        m = dict(shared)
        m["xq"] = _xtiles(xq_c)
        m["xk"] = _xtiles(_xT_padded(key[b]))
        m["xv"] = _xtiles(_xT_padded(value[b]))
        in_maps.append(m)

    nc = _get_nc()
    res = run_bass_kernel_spmd(nc, in_maps, core_ids=list(range(NCORES)))

    out_full = np.empty((B, S, D), np.float32)
    for c in range(NCORES):
        b, h = c // 2, c % 2
        out_full[b, h * SQ : (h + 1) * SQ, :] = res.results[c]["out"].reshape(SQ, D)
    return out_full
